# revision 26
# baseline (speedup 1.0000x reference)
"""DividedAttention (TimeSformer-style divided space-time attention) on 8 trn2 cores.

Sharding: pure data-parallel over batch B=16 -> 2 batch items per core.
Per-core pipeline (bf16 matmuls, fp32 accumulation):
  host: x -> xT (pre-transposed, bf16), W_qkv (q part pre-scaled by dh^-0.5), W_out
  V projection first (natural form) -> v_flat -> frame-aligned v_fra/v_frb
    (rearrange DMAs ride the gpsimd queue; weights too — keeps the sync queue
    free for xT/output DMAs, each dma_start costs ~0.8us of its queue)
  Q/K projection (out^T form), ci-block-major so attention can start after ci=0
  attention f-major in head-pair "pairs": S^T (row-tiled parity mms; parity on
    the 2KB psum stride — concurrent tiled mms must hit different psum banks)
    -> one exp ACTIVATE per step (kT zero-padded so chunk-b is M=128, no
    uninit psum reads) -> AV + ones-broadcast sums (col-tiled parity mms) into
    a per-pair psum tile -> pair-batched DVE: reciprocal, normalize-mult into
    attnT, cls accumulate.  CLS rides as q column 196; corrected at end.
  Q/K ci=1..3 chains interleave into the attention pair loop as PE filler with
  just-in-time deadlines (ci feeds frames 2ci,2ci+1 at pair 4ci); item-1's V
  pipeline threads through item-0's attention the same way; item-0's
  out-projection interleaves into item-1's projection head; item-1's follows
  frame completion.  This keeps the PE ~95% busy so the HAM clock-gate stays
  at 2.4GHz.  Output stored bf16, upcast on host.
"""
import sys

sys.path.insert(0, "/opt/trn_rl_repo")

import numpy as np
import ml_dtypes

from concourse import bacc
import concourse.mybir as mybir
import concourse.tile as tile
from concourse import bass_utils

BF16 = mybir.dt.bfloat16
F32 = mybir.dt.float32
NPBF = ml_dtypes.bfloat16

B, SP, F, DIM, H, DH = 16, 196, 8, 512, 8, 64
INNER = H * DH            # 512
N = 1 + F * SP            # 1569
SP1 = SP + 1              # 197
NCORES = 8
NB = B // NCORES          # 2
KC = DIM // 128           # 4
NT = (N + 127) // 128     # 13
LAST = N - 128 * (NT - 1)  # 33
TCH = [(0, 1 + 2 * SP), (1 + 2 * SP, 2 * SP), (1 + 4 * SP, 2 * SP), (1 + 6 * SP, 2 * SP)]

# out-proj token tile t is ready once frame FMAX_T[t] is normalized (t=0 needs CLS, kept last)
FMAX_T = {t: min((128 * (t + 1) - 2) // SP, F - 1) for t in range(1, NT)}

EXP = mybir.ActivationFunctionType.Exp
ADD = mybir.AluOpType.add
MULT = mybir.AluOpType.mult

import os
INTERLEAVE = os.environ.get("K_INTERLEAVE", "1") == "1"
ONE_ACT = os.environ.get("K_ONE_ACT", "1") == "1"
ONE_RECIP = os.environ.get("K_ONE_RECIP", "1") == "1"
OUT_BF16 = os.environ.get("K_OUT_BF16", "1") == "1"
ODT = BF16 if OUT_BF16 else F32


def _v_pieces(tok0, length):
    out = []
    done = 0
    while done < length:
        tok = tok0 + done
        t, p0 = divmod(tok, 128)
        l = min(128 - p0, length - done)
        out.append((t, p0, done, l))
        done += l
    return out


def build_nc():
    nc = bacc.Bacc(num_devices=NCORES)

    xT = nc.declare_dram_parameter("xT", [NB, DIM, N], BF16, isOutput=False)
    wqkv = nc.declare_dram_parameter("wqkv", [DIM, 3 * INNER], BF16, isOutput=False)
    wout = nc.declare_dram_parameter("wout", [INNER, DIM], BF16, isOutput=False)
    bout = nc.declare_dram_parameter("bout", [1, DIM], F32, isOutput=False)
    out = nc.declare_dram_parameter("out", [NB, N, DIM], ODT, isOutput=True)

    with tile.TileContext(nc) as tc:
        with (
            tc.tile_pool(name="const", bufs=1) as const,
            tc.tile_pool(name="big", bufs=1) as big,
            tc.tile_pool(name="big2", bufs=2) as big2,
            tc.tile_pool(name="ebuf", bufs=6) as ebuf,
            tc.tile_pool(name="small", bufs=4) as small,
            tc.tile_pool(name="outp", bufs=3) as outp,
            tc.tile_pool(name="ps_proj", bufs=2, space="PSUM") as ps_proj,
            tc.tile_pool(name="ps_s", bufs=2, space="PSUM") as ps_s,
            tc.tile_pool(name="ps_o", bufs=1, space="PSUM") as ps_o,
        ):
            # ---- constants on the gpsimd queue (parallel with xT on sync;
            # v part of wqkv first: V-projection runs first)
            wqkv_sb = const.tile([128, KC, 3 * INNER], BF16)
            nc.gpsimd.dma_start(
                out=wqkv_sb[:, :, 2 * INNER:3 * INNER],
                in_=wqkv[:, 2 * INNER:3 * INNER].rearrange("(c p) o -> p c o", p=128))
            nc.gpsimd.dma_start(
                out=wqkv_sb[:, :, 0:INNER],
                in_=wqkv[:, 0:INNER].rearrange("(c p) o -> p c o", p=128))
            nc.gpsimd.dma_start(
                out=wqkv_sb[:, :, INNER:2 * INNER],
                in_=wqkv[:, INNER:2 * INNER].rearrange("(c p) o -> p c o", p=128))
            wout_sb = const.tile([128, KC, DIM], BF16)
            nc.gpsimd.dma_start(out=wout_sb, in_=wout[:, :].rearrange("(c p) o -> p c o", p=128))
            bout_sb = const.tile([128, DIM], F32)
            nc.gpsimd.dma_start(out=bout_sb, in_=bout[:, :].to_broadcast([128, DIM]))
            ones_row = const.tile([1, 64], BF16)
            nc.vector.memset(ones_row, 1.0)
            ones128 = const.tile([128, 64], BF16)
            nc.vector.memset(ones128, 1.0)

            # ---- xT loads, t-chunk-major, one 3D DMA per chunk (all kc),
            # striped across the sync and vector DMA queues so chunks land in
            # parallel; both items up front (b1 prefetches behind b0 and
            # lands during b0 compute)
            xts = []
            for b in range(NB):
                xT_sb = big2.tile([128, KC, N], BF16, tag="xt")
                for ci, (t0, tl) in enumerate(TCH):
                    eng = nc.sync if ci % 2 == 0 else nc.scalar
                    eng.dma_start(
                        out=xT_sb[:, :, t0:t0 + tl],
                        in_=xT[b, :, t0:t0 + tl].rearrange("(c p) t -> p c t", p=128))
                xts.append(xT_sb)

            # =============== per-item emitters ===============

            # v-rearrange DMA pieces grouped by the last v_flat tile they read
            VPIECES = {t: [] for t in range(NT)}
            for f in range(F):
                for (t, p0, d0, l) in _v_pieces(1 + SP * f, 127):
                    VPIECES[t].append(("a", f, t, p0, d0, l))
                for (t, p0, d0, l) in _v_pieces(128 + SP * f, 69):
                    VPIECES[t].append(("b", f, t, p0, d0, l))

            def v_tile_unit(xT_sb, v_flat, v_fra, v_frb, t):
                """One V-proj token tile: chain + copy + the rearrange DMAs
                whose source tile this completes.  DMAs ride the gpsimd queue
                so they don't jam the sync queue's out-projection DMAs."""
                m = 128 if t < NT - 1 else LAST
                ps = ps_proj.tile([128, 512], F32, tag="proj")
                for kc in range(KC):
                    nc.tensor.matmul(
                        ps[:m, :],
                        lhsT=xT_sb[:, kc, 128 * t:128 * t + m],
                        rhs=wqkv_sb[:, kc, 2 * INNER:3 * INNER],
                        start=(kc == 0), stop=(kc == KC - 1))
                if t % 2 == 0:
                    nc.scalar.copy(v_flat[:m, t, :], ps[:m, :])
                else:
                    nc.vector.tensor_copy(v_flat[:m, t, :], ps[:m, :])
                if t == 0:
                    nc.gpsimd.dma_start(
                        out=v_fra[0:1, 0:F, :],
                        in_=v_flat[0:1, 0, None, :].to_broadcast([1, F, INNER]))
                for (which, f, tt, p0, d0, l) in VPIECES[t]:
                    if which == "a":
                        nc.gpsimd.dma_start(
                            out=v_fra[1 + d0:1 + d0 + l, f, :],
                            in_=v_flat[p0:p0 + l, tt, :])
                    else:
                        nc.gpsimd.dma_start(
                            out=v_frb[d0:d0 + l, f, :],
                            in_=v_flat[p0:p0 + l, tt, :])

            def v_finish(v_flat, vTcls):
                # vT_cls [d x 2heads, hp] via tiny K=1 transposes of the V cls row
                pvt = ps_proj.tile([128, 512], F32, tag="proj")
                for hp in range(4):
                    nc.tensor.matmul(
                        pvt[:, hp:hp + 1],
                        lhsT=v_flat[0:1, 0, 128 * hp:128 * (hp + 1)],
                        rhs=ones_row[0:1, 0:1],
                        start=True, stop=True)
                nc.vector.tensor_copy(vTcls, pvt[:, 0:4])

            def v_proj(b, xT_sb, v_flat, v_fra, v_frb, vTcls):
                for t in range(NT):
                    v_tile_unit(xT_sb, v_flat, v_fra, v_frb, t)
                v_finish(v_flat, vTcls)

            def qk_chain(xT_sb, qT_fr, kT_fr, ci, oc):
                t0, tl = TCH[ci]
                isq = oc < 4
                hp = oc if isq else oc - 4
                ps = ps_proj.tile([128, 512], F32, tag="proj")
                for kc in range(KC):
                    nc.tensor.matmul(
                        ps[:, :tl],
                        lhsT=wqkv_sb[:, kc, oc * 128:(oc + 1) * 128],
                        rhs=xT_sb[:, kc, t0:t0 + tl],
                        start=(kc == 0), stop=(kc == KC - 1))
                dst = qT_fr if isq else kT_fr
                eng = nc.scalar if isq else nc.vector
                cp = eng.copy if isq else eng.tensor_copy
                o0 = 0 if isq else 1
                if ci == 0:
                    cp(dst[:, hp, 0:2, o0:o0 + SP],
                       ps[:, 1:tl].rearrange("p (a s) -> p a s", a=2))
                    ccol = SP if isq else 0
                    cp(dst[:, hp, 0:F, ccol:ccol + 1],
                       ps[:, None, 0:1].to_broadcast([128, F, 1]))
                else:
                    cp(dst[:, hp, 2 * ci:2 * ci + 2, o0:o0 + SP],
                       ps[:, :tl].rearrange("p (a s) -> p a s", a=2))

            def outproj_chain(b, t, attnT_b):
                m = 128 if t < NT - 1 else LAST
                ps = ps_proj.tile([128, 512], F32, tag="proj")
                for kc in range(KC):
                    nc.tensor.matmul(
                        ps[:m, :],
                        lhsT=attnT_b[:, kc, 128 * t:128 * t + m],
                        rhs=wout_sb[:, kc, :],
                        start=(kc == 0), stop=(kc == KC - 1))
                osb = outp.tile([128, DIM], ODT, tag="out")
                nc.vector.tensor_tensor(osb[:m, :], ps[:m, :], bout_sb[:m, :], ADD)
                nc.sync.dma_start(out=out[b, 128 * t:128 * t + m, :], in_=osb[:m, :])

            def attention(b, qT_fr, kT_fr, v_fra, v_frb, attnT_b,
                          oscls_acc, ecc_row, fillers, post_pair_fillers):
                """fillers: list of (due_pair, callable); each callable MUST be
                emitted before pass1 of pair `due_pair` (use a large due for
                no deadline).  Paced: deadlines first, then proportionally.
                post_pair_fillers: dict pair_idx -> list of callables emitted
                after that pair's pass2 (used to emit outproj of ready frames).
                """
                def pass1(f, hp):
                    # st layout [128, par, chunk, 256]: the row-tiled parity
                    # mms run CONCURRENTLY on the PE, so their outputs must
                    # land in different PSUM banks (2KB par stride); the two
                    # chunk mms of one parity share a bank but serialize
                    # (same PE row group).
                    st = ps_s.tile([128, 2, 2, 256], F32, tag="s")
                    for ch in range(2):  # k chunks: a=tokens 0..127, b=128..196(+pad)
                        c0 = 128 * ch
                        for par in range(2):
                            rows = slice(64 * par, 64 * par + 64)
                            nc.tensor.matmul(
                                st[:, par, ch, 0:SP1],
                                lhsT=kT_fr[rows, hp, f, c0:c0 + 128],
                                rhs=qT_fr[rows, hp, f, :],
                                start=True, stop=True)
                    e = ebuf.tile([128, 2, 2, SP1], BF16, tag="e")
                    if ONE_ACT:
                        nc.scalar.activation(e, st[:, :, :, 0:SP1], EXP)
                    else:
                        nc.scalar.activation(e[:, 0, :, :], st[:, 0, :, 0:SP1], EXP)
                        nc.scalar.activation(e[:, 1, :, :], st[:, 1, :, 0:SP1], EXP)
                    if f == 0:
                        for par in range(2):
                            h = 2 * hp + par
                            nc.vector.tensor_copy(ecc_row[0:1, h:h + 1],
                                                  e[0:1, par, 0, SP:SP1])
                    return e

                def pass2_mms(f, hp, po, slot, e):
                    for par in range(2):
                        h = 2 * hp + par
                        rows = slice(64 * par, 64 * par + 64)
                        hs = slice(DH * h, DH * (h + 1))
                        nc.tensor.matmul(
                            po[rows, slot, 0:SP1],
                            lhsT=v_fra[:, f, hs], rhs=e[:, par, 0, :],
                            start=True, stop=False)
                        nc.tensor.matmul(
                            po[rows, slot, 0:SP1],
                            lhsT=v_frb[0:69, f, hs], rhs=e[0:69, par, 1, :],
                            start=False, stop=True)
                    for par in range(2):
                        rows = slice(64 * par, 64 * par + 64)
                        nc.tensor.matmul(
                            po[rows, slot, 256:256 + SP1],
                            lhsT=ones128[:, 0:64], rhs=e[:, par, 0, :],
                            start=True, stop=False)
                        nc.tensor.matmul(
                            po[rows, slot, 256:256 + SP1],
                            lhsT=ones128[0:69, 0:64], rhs=e[0:69, par, 1, :],
                            start=False, stop=True)

                def pass2_dve(f, qh, po):
                    rbc = small.tile([128, 2, SP1], F32, tag="rbc")
                    if ONE_RECIP:
                        nc.vector.reciprocal_approx_fast(rbc, po[:, :, 256:256 + SP1])
                    else:
                        nc.vector.reciprocal_approx_fast(rbc[:, 0, :], po[:, 0, 256:256 + SP1])
                        nc.vector.reciprocal_approx_fast(rbc[:, 1, :], po[:, 1, 256:256 + SP1])
                    nc.vector.tensor_tensor(
                        attnT_b[:, 2 * qh:2 * qh + 2, 1 + SP * f:1 + SP * (f + 1)],
                        po[:, :, 0:SP], rbc[:, :, 0:SP], MULT)
                    nc.vector.tensor_tensor(
                        oscls_acc[:, 2 * qh:2 * qh + 2, :],
                        po[:, :, SP:SP + 257:256],
                        oscls_acc[:, 2 * qh:2 * qh + 2, :], ADD)

                pairs = [(f, qh) for f in range(F) for qh in range(2)]
                LAGP = 2
                es = {}
                fi = 0
                for i in range(len(pairs) + LAGP):
                    if i < len(pairs):
                        f, qh = pairs[i]
                        es[i] = (pass1(f, 2 * qh), pass1(f, 2 * qh + 1))
                        # interleave filler chains (PE-dense proj work):
                        # anything due before pass1(i+1) first, then pace the
                        # rest proportionally across the pair loop
                        target = (i + 1) * len(fillers) // len(pairs)
                        while fi < len(fillers) and (
                                fillers[fi][0] <= i + 1 or fi < target):
                            fillers[fi][1]()
                            fi += 1
                    if i >= LAGP:
                        j = i - LAGP
                        f, qh = pairs[j]
                        po = ps_o.tile([128, 2, 512], F32, tag="po")
                        e0, e1 = es.pop(j)
                        pass2_mms(f, 2 * qh, po, 0, e0)
                        pass2_mms(f, 2 * qh + 1, po, 1, e1)
                        pass2_dve(f, qh, po)
                        for fn in post_pair_fillers.get(j, ()):
                            fn()
                while fi < len(fillers):
                    fillers[fi][1]()
                    fi += 1

            def cls_finalize(attnT_b, oscls_acc, ecc_row, vTcls,
                             ecc_bc, rcls_bc, t_evc, t_corr):
                ocls_acc = oscls_acc[:, :, 0]
                scls_acc = oscls_acc[:, :, 1]
                pec = ps_proj.tile([128, 512], F32, tag="proj")
                for hp in range(4):
                    for par in range(2):
                        h = 2 * hp + par
                        rows = slice(64 * par, 64 * par + 64)
                        nc.tensor.matmul(pec[rows, hp:hp + 1], lhsT=ones_row,
                                         rhs=ecc_row[0:1, h:h + 1],
                                         start=True, stop=True)
                nc.vector.tensor_copy(ecc_bc, pec[:, 0:4])
                # denominator: scls_acc - 7 e_cc -> reciprocal
                nc.vector.scalar_tensor_tensor(
                    scls_acc, ecc_bc, -7.0, scls_acc, op0=MULT, op1=ADD)
                nc.vector.reciprocal_approx_fast(rcls_bc, scls_acc)
                # numerator: ocls_acc - 7 e_cc * vTcls, then normalize
                nc.vector.tensor_tensor(t_evc, ecc_bc, vTcls, MULT)
                nc.vector.scalar_tensor_tensor(
                    t_corr, t_evc, -7.0, ocls_acc, op0=MULT, op1=ADD)
                nc.vector.tensor_tensor(t_corr, t_corr, rcls_bc, MULT)
                nc.vector.tensor_copy(attnT_b[:, 0:4, 0:1], t_corr[:, :, None])

            # =============== program ===============
            # allocate both items' big tiles up front (big2 pools: two live
            # generations; b1's pipeline threads through b0's attention)
            items = []
            for b in range(NB):
                it = {
                    "xT": xts[b],
                    "qT": big2.tile([128, 4, F, SP1], BF16, tag="qT", name=f"qT{b}"),
                    "kT": big2.tile([128, 4, F, 256], BF16, tag="kT", name=f"kT{b}"),
                    "vflat": big.tile([128, NT, INNER], BF16, tag="vflat", name=f"vflat{b}"),
                    "vfra": big2.tile([128, F, INNER], BF16, tag="vfra", name=f"vfra{b}"),
                    "vfrb": big2.tile([128, F, INNER], BF16, tag="vfrb", name=f"vfrb{b}"),
                    "attnT": big2.tile([128, KC, N], BF16, tag="attnT", name=f"attnT{b}"),
                    "vTcls": big2.tile([128, 4], F32, tag="vTcls", name=f"vTcls{b}"),
                }
                items.append(it)

            for b in range(NB):
                it = items[b]
                xT_sb, qT_fr, kT_fr = it["xT"], it["qT"], it["kT"]
                v_flat, v_fra, v_frb = it["vflat"], it["vfra"], it["vfrb"]
                attnT_b, vTcls = it["attnT"], it["vTcls"]
                oscls_acc = big.tile([128, 4, 2], F32, tag="oclsacc")
                ecc_bc = big.tile([128, 4], F32, tag="eccbc")
                ecc_row = big.tile([1, H], BF16, tag="eccrow")
                rcls_bc = big.tile([128, 4], F32, tag="rclsbc")
                t_evc = big.tile([128, 4], F32, tag="tevc")
                t_corr = big.tile([128, 4], F32, tag="tcorr")

                nc.vector.memset(oscls_acc, 0.0)
                nc.vector.memset(kT_fr[:, :, :, SP1:256], 0.0)

                if b == 0:
                    # projection head: V first, then Q/K ci=0 block
                    v_proj(b, xT_sb, v_flat, v_fra, v_frb, vTcls)
                    for oc in range(8):
                        qk_chain(xT_sb, qT_fr, kT_fr, 0, oc)
                else:
                    # V pipeline already ran as b0-attention fillers.
                    # b0's outproj interleaves into this projection head.
                    head_fillers = [
                        (lambda t=t: outproj_chain(0, t, items[0]["attnT"]))
                        for t in list(range(1, NT)) + [0]
                    ]
                    hf = 0
                    for oc in range(8):
                        qk_chain(xT_sb, qT_fr, kT_fr, 0, oc)
                        while hf < len(head_fillers) and hf < 2 * (oc + 1):
                            head_fillers[hf]()
                            hf += 1
                    while hf < len(head_fillers):
                        head_fillers[hf]()
                        hf += 1

                # attention-phase fillers: this item's remaining Q/K chains
                # (ci feeds f=2ci,2ci+1 just in time); for b0 additionally
                # the whole V pipeline of b1
                fillers = [
                    (4 * ci, (lambda ci=ci, oc=oc: qk_chain(xT_sb, qT_fr, kT_fr, ci, oc)))
                    for ci in range(1, KC) for oc in range(8)
                ]
                if b == 0:
                    n1 = items[1]
                    fillers += [
                        (99, (lambda t=t: v_tile_unit(n1["xT"], n1["vflat"],
                                                      n1["vfra"], n1["vfrb"], t)))
                        for t in range(NT)
                    ]
                    fillers.append((99, lambda: v_finish(n1["vflat"], n1["vTcls"])))
                    # keep deadline-ordered emission monotone: ci chains carry
                    # due 4/8/12, v units 99 — already sorted
                post = {}
                if not INTERLEAVE:
                    for _, fn in fillers:
                        fn()
                    fillers = []
                # out-projection of this item's ready frames (b=1 only; b=0's
                # outproj runs in b=1's projection head, where DVE has slack)
                if b == 1 and INTERLEAVE:
                    for t, fmax in FMAX_T.items():
                        # pair index of (f=fmax, qh=1)
                        post.setdefault(2 * fmax + 1, []).append(
                            lambda t=t: outproj_chain(1, t, attnT_b))

                attention(b, qT_fr, kT_fr, v_fra, v_frb, attnT_b,
                          oscls_acc, ecc_row, fillers, post)
                cls_finalize(attnT_b, oscls_acc, ecc_row, vTcls,
                             ecc_bc, rcls_bc, t_evc, t_corr)
                if b == 1 or not INTERLEAVE:
                    ts = (list(range(1, NT)) + [0]) if not INTERLEAVE else [0]
                    for t in ts:
                        outproj_chain(b, t, attnT_b)

    nc.finalize()
    return nc


_CACHE = {}


def _get_nc():
    if "nc" not in _CACHE:
        _CACHE["nc"] = build_nc()
    return _CACHE["nc"]


def prepare_in_maps(x, f, W_qkv, W_out, b_out):
    assert int(f) == F
    x = np.asarray(x, dtype=np.float32)
    W_qkv = np.asarray(W_qkv, dtype=np.float32).copy()
    W_out = np.asarray(W_out, dtype=np.float32)
    b_out = np.asarray(b_out, dtype=np.float32)
    W_qkv[:, :INNER] *= DH ** -0.5
    wqkv_bf = W_qkv.astype(NPBF)
    wout_bf = W_out.astype(NPBF)
    bout_np = b_out.reshape(1, DIM)
    xT = np.ascontiguousarray(x.transpose(0, 2, 1)).astype(NPBF)
    in_maps = []
    for c in range(NCORES):
        in_maps.append({
            "xT": np.ascontiguousarray(xT[NB * c:NB * (c + 1)]),
            "wqkv": wqkv_bf,
            "wout": wout_bf,
            "bout": bout_np,
        })
    return in_maps


def kernel(x, f, W_qkv, W_out, b_out):
    nc = _get_nc()
    in_maps = prepare_in_maps(x, f, W_qkv, W_out, b_out)
    res = bass_utils.run_bass_kernel_spmd(nc, in_maps, list(range(NCORES)))
    return np.concatenate(
        [np.asarray(r["out"], dtype=np.float32) for r in res.results], axis=0)
